# revision 4
# baseline (speedup 1.0000x reference)
"""GroupedQueryAttention Trainium2 kernel (8-core SPMD).

Sharding: core c -> batch b = c//4, head-group j = c%4
  (query heads 8j..8j+8, kv heads 2j, 2j+1). w_q/w_k/w_v column-parallel,
  w_o row-parallel with host-side partial-sum reduction.

Per-core device program (all matmuls in fp32r, full-rate on PE):
  P1: Q/K/V projections in transposed layout (features on partitions),
      K_T duplicated to both partition halves (matmul base-partition
      alignment), V_T transposed back to [pos, feat] tiles via PE.
  P2: per q-head, scores both orientations; natural side does
      exp(+accum row-sums) -> normalize -> DMA attention weights out;
      transposed side feeds unnormalized exp into the attnV matmul,
      normalized from the natural-side reciprocals broadcast via
      indicator matmuls.
  P3: output projection from attnV^T tiles; host sums the 4 per-core
      partials per batch.
"""

import numpy as np

D = 2048
S = 2048
B = 2
HQ = 32
HKV = 8
G = 4
DK = 64
NCORES = 8
NH = 8      # q-heads per core
NKV = 2     # kv-heads per core
FEAT = NH * DK  # 512 q-features per core

_CACHE = {}


def _build_nc():
    import concourse.bass as bass  # noqa: F401
    import concourse.mybir as mybir
    import concourse.tile as tile
    from concourse import bacc
    from contextlib import ExitStack

    f32 = mybir.dt.float32
    f32r = mybir.dt.float32r
    EXP = mybir.ActivationFunctionType.Exp

    nc = bacc.Bacc()
    qT = nc.dram_tensor("qT", [D, S], f32r, kind="ExternalInput")
    kT = nc.dram_tensor("kT", [D, S], f32r, kind="ExternalInput")
    vT = nc.dram_tensor("vT", [D, S], f32r, kind="ExternalInput")
    wqT = nc.dram_tensor("wqT", [D, FEAT], f32r, kind="ExternalInput")
    wkT = nc.dram_tensor("wkT", [D, NKV * DK], f32r, kind="ExternalInput")
    wvT = nc.dram_tensor("wvT", [D, NKV * DK], f32r, kind="ExternalInput")
    woT = nc.dram_tensor("woT", [FEAT, D], f32r, kind="ExternalInput")
    ind = nc.dram_tensor("ind", [16, 1024], f32r, kind="ExternalInput")
    idn_f = nc.dram_tensor("idn_f", [128, 128], f32, kind="ExternalInput")
    idn_r = nc.dram_tensor("idn_r", [128, 128], f32r, kind="ExternalInput")
    attn = nc.dram_tensor("attn", [NH, S, S], f32, kind="ExternalOutput")
    outp = nc.dram_tensor("outp", [S, D], f32, kind="ExternalOutput")

    with tile.TileContext(nc) as tc, ExitStack() as top:
        pers = top.enter_context(tc.tile_pool(name="pers", bufs=1))
        ident = pers.tile([128, 128], f32)
        nc.sync.dma_start(out=ident, in_=idn_f[:, :])
        ind_sb = pers.tile([16, 1024], f32r)
        nc.sync.dma_start(out=ind_sb, in_=ind[:, :])
        identr = pers.tile([128, 128], f32r)
        nc.sync.dma_start(out=identr, in_=idn_r[:, :])

        QT = [pers.tile([128, S], f32r, tag=f"QT{i}", name=f"QT{i}") for i in range(4)]
        KTD = [pers.tile([128, S], f32r, tag=f"KTD{i}", name=f"KTD{i}") for i in range(2)]
        vns = [pers.tile([128, 128], f32r, tag=f"vns{i}", name=f"vns{i}") for i in range(16)]
        avT = [pers.tile([128, S], f32r, tag=f"avT{i}", name=f"avT{i}") for i in range(4)]
        recnat = [pers.tile([128, 16], f32, tag=f"rn{i}", name=f"rn{i}") for i in range(NH)]

        # ---------------- Phase 1: projections ----------------
        with ExitStack() as ph1:
            wpool = ph1.enter_context(tc.tile_pool(name="wts", bufs=1))
            wq_sb = wpool.tile([128, 16, FEAT], f32r)
            nc.sync.dma_start(out=wq_sb, in_=wqT.rearrange("(kt p) f -> p kt f", p=128))
            wk_sb = wpool.tile([128, 16, 128], f32r)
            nc.sync.dma_start(out=wk_sb, in_=wkT.rearrange("(kt p) f -> p kt f", p=128))
            wv_sb = wpool.tile([128, 16, 128], f32r)
            nc.sync.dma_start(out=wv_sb, in_=wvT.rearrange("(kt p) f -> p kt f", p=128))
            KTtmp = wpool.tile([128, S], f32r)
            VTtmp = wpool.tile([128, S], f32r)

            inp = ph1.enter_context(tc.tile_pool(name="inp", bufs=3))
            qT_r = qT.rearrange("(kc two p) (qb f) -> p two kc qb f",
                                two=2, p=128, f=512)
            kT_r = kT.rearrange("(kc two p) (qb f) -> p two kc qb f",
                                two=2, p=128, f=512)
            vT_r = vT.rearrange("(kc two p) (qb f) -> p two kc qb f",
                                two=2, p=128, f=512)
            with ExitStack() as ph1a:
                psp = ph1a.enter_context(
                    tc.tile_pool(name="psp", bufs=1, space="PSUM"))
                for qb in range(4):
                    psq = [psp.tile([128, 512], f32, tag=f"psq{ft}", name=f"psq{ft}")
                           for ft in range(4)]
                    psk = psp.tile([128, 512], f32, tag="psk")
                    psv = psp.tile([128, 512], f32, tag="psv")
                    for kc in range(8):
                        qc = inp.tile([128, 2, 512], f32r, tag="qc")
                        nc.sync.dma_start(out=qc, in_=qT_r[:, :, kc, qb, :])
                        kc_ = inp.tile([128, 2, 512], f32r, tag="kc")
                        nc.sync.dma_start(out=kc_, in_=kT_r[:, :, kc, qb, :])
                        vc = inp.tile([128, 2, 512], f32r, tag="vc")
                        nc.sync.dma_start(out=vc, in_=vT_r[:, :, kc, qb, :])
                        for two in range(2):
                            kt = kc * 2 + two
                            st = dict(start=(kt == 0), stop=(kt == 15))
                            for ft in range(4):
                                nc.tensor.matmul(
                                    psq[ft],
                                    wq_sb[:, kt, ft * 128:(ft + 1) * 128],
                                    qc[:, two, :], **st)
                            nc.tensor.matmul(psk, wk_sb[:, kt, :],
                                             kc_[:, two, :], **st)
                            nc.tensor.matmul(psv, wv_sb[:, kt, :],
                                             vc[:, two, :], **st)
                    sl = slice(qb * 512, (qb + 1) * 512)
                    for ft in range(4):
                        nc.vector.tensor_copy(QT[ft][:, sl], psq[ft])
                    nc.vector.tensor_copy(KTtmp[:, sl], psk)
                    nc.vector.tensor_copy(VTtmp[:, sl], psv)
                # duplicate K_T halves for base-partition alignment
                nc.sync.dma_start(out=KTD[0][0:64, :], in_=KTtmp[0:64, :])
                nc.sync.dma_start(out=KTD[0][64:128, :], in_=KTtmp[0:64, :])
                nc.sync.dma_start(out=KTD[1][0:64, :], in_=KTtmp[64:128, :])
                nc.sync.dma_start(out=KTD[1][64:128, :], in_=KTtmp[64:128, :])
            with ExitStack() as ph1b:
                pst = ph1b.enter_context(
                    tc.tile_pool(name="pst", bufs=2, space="PSUM"))
                for c in range(16):
                    pvt = pst.tile([128, 128], f32r, tag="pvt")
                    nc.tensor.matmul(pvt, VTtmp[:, c * 128:(c + 1) * 128],
                                     identr, is_transpose=True,
                                     start=True, stop=True)
                    nc.vector.tensor_copy(vns[c], pvt)

        # ---------------- Phase 2: attention ----------------
        for lpair in (0, 2, 4, 6):
            kvh = lpair // 4
            # natural orientation: attn weights out + row-sum reciprocals
            with ExitStack() as sn:
                pnp = sn.enter_context(
                    tc.tile_pool(name="pn", bufs=2, space="PSUM"))
                enp = sn.enter_context(tc.tile_pool(name="en", bufs=2))
                anp = sn.enter_context(tc.tile_pool(name="an", bufs=3))
                for qt in range(16):
                    for loff in (lpair, lpair + 1):
                        p64 = 64 * (loff % 2)
                        tq = loff // 2
                        pn = pnp.tile([128, 2048], f32, tag="pn")
                        for kb in range(4):
                            nc.tensor.matmul(
                                pn[:, kb * 512:(kb + 1) * 512],
                                QT[tq][p64:p64 + 64, qt * 128:(qt + 1) * 128],
                                KTD[kvh][p64:p64 + 64, kb * 512:(kb + 1) * 512],
                                start=True, stop=True)
                        en = enp.tile([128, 2048], f32, tag="en")
                        sums = enp.tile([128, 1], f32, tag="sums")
                        nc.scalar.activation(out=en, in_=pn, func=EXP,
                                             scale=0.125, accum_out=sums)
                        nc.vector.reciprocal(
                            recnat[loff][:, qt:qt + 1], sums)
                        an = anp.tile([128, 2048], f32, tag="an")
                        nc.vector.tensor_scalar_mul(
                            an, en, recnat[loff][:, qt:qt + 1])
                        nc.sync.dma_start(
                            out=attn[loff, qt * 128:(qt + 1) * 128, :], in_=an)
            # transposed orientation: attnV
            for loff in (lpair, lpair + 1):
                p64 = 64 * (loff % 2)
                tq = loff // 2
                with ExitStack() as st_:
                    rbcp = st_.enter_context(tc.tile_pool(name="rbc", bufs=1))
                    rbc = rbcp.tile([64, 2048], f32)
                    with ExitStack() as spro:
                        ptp0 = spro.enter_context(
                            tc.tile_pool(name="ptr", bufs=1, space="PSUM"))
                        pbcp = spro.enter_context(
                            tc.tile_pool(name="pbc", bufs=2, space="PSUM"))
                        ptr = ptp0.tile([16, 128], f32)
                        nc.tensor.matmul(ptr, recnat[loff], ident,
                                         is_transpose=True,
                                         start=True, stop=True)
                        rTq = rbcp.tile([16, 128], f32r, tag="rTq")
                        nc.vector.tensor_copy(rTq, ptr)
                        for qt in range(16):
                            pbc = pbcp.tile([64, 128], f32, tag="pbc")
                            nc.tensor.matmul(
                                pbc, ind_sb[:, qt * 64:(qt + 1) * 64], rTq,
                                start=True, stop=True)
                            nc.vector.tensor_copy(
                                rbc[:, qt * 128:(qt + 1) * 128], pbc)
                    with ExitStack() as skv:
                        ptp = skv.enter_context(
                            tc.tile_pool(name="pt", bufs=1, space="PSUM"))
                        pavp = skv.enter_context(
                            tc.tile_pool(name="pav", bufs=1, space="PSUM"))
                        etp = skv.enter_context(tc.tile_pool(name="et", bufs=2))
                        pavs = [pavp.tile([64, 512], f32, tag=f"pav{qb}", name=f"pav{qb}")
                                for qb in range(4)]
                        for kvt in range(16):
                            pt = ptp.tile([128, 2048], f32, tag="pt")
                            for qb in range(4):
                                nc.tensor.matmul(
                                    pt[:, qb * 512:(qb + 1) * 512],
                                    KTD[kvh][p64:p64 + 64,
                                             kvt * 128:(kvt + 1) * 128],
                                    QT[tq][p64:p64 + 64,
                                           qb * 512:(qb + 1) * 512],
                                    start=True, stop=True)
                            et = etp.tile([128, 2048], f32r, tag="et")
                            nc.scalar.activation(out=et, in_=pt, func=EXP,
                                                 scale=0.125)
                            for qb in range(4):
                                nc.tensor.matmul(
                                    pavs[qb],
                                    vns[kvt][:, 64 * kvh:64 * kvh + 64],
                                    et[:, qb * 512:(qb + 1) * 512],
                                    start=(kvt == 0), stop=(kvt == 15))
                        for qb in range(4):
                            nc.vector.tensor_mul(
                                avT[tq][p64:p64 + 64,
                                        qb * 512:(qb + 1) * 512],
                                pavs[qb], rbc[:, qb * 512:(qb + 1) * 512])

        # ---------------- Phase 3: output projection ----------------
        with ExitStack() as ph3:
            wop = ph3.enter_context(tc.tile_pool(name="wo", bufs=1))
            wo_sb = wop.tile([128, 4, 2048], f32r)
            nc.sync.dma_start(out=wo_sb,
                              in_=woT.rearrange("(ft p) m -> p ft m", p=128))
            pso = ph3.enter_context(
                tc.tile_pool(name="pso", bufs=4, space="PSUM"))
            otp = ph3.enter_context(tc.tile_pool(name="ot", bufs=3))
            for pt_ in range(16):
                ot = otp.tile([128, 2048], f32, tag="ot")
                for ob in range(4):
                    po = pso.tile([128, 512], f32, tag="po")
                    for ft in range(4):
                        nc.tensor.matmul(
                            po, avT[ft][:, pt_ * 128:(pt_ + 1) * 128],
                            wo_sb[:, ft, ob * 512:(ob + 1) * 512],
                            start=(ft == 0), stop=(ft == 3))
                    nc.vector.tensor_copy(ot[:, ob * 512:(ob + 1) * 512], po)
                nc.sync.dma_start(out=outp[pt_ * 128:(pt_ + 1) * 128, :],
                                  in_=ot)
    nc.finalize()
    return nc


def _get_nc():
    if "nc" not in _CACHE:
        _CACHE["nc"] = _build_nc()
    return _CACHE["nc"]


def _prep_in_maps(query, key, value, w_q, w_k, w_v, w_o):
    query = np.asarray(query, dtype=np.float32)
    key = np.asarray(key, dtype=np.float32)
    value = np.asarray(value, dtype=np.float32)
    w_q = np.asarray(w_q, dtype=np.float32)
    w_k = np.asarray(w_k, dtype=np.float32)
    w_v = np.asarray(w_v, dtype=np.float32)
    w_o = np.asarray(w_o, dtype=np.float32)

    qT = [np.ascontiguousarray(query[b].T) for b in range(B)]
    kTb = [np.ascontiguousarray(key[b].T) for b in range(B)]
    vTb = [np.ascontiguousarray(value[b].T) for b in range(B)]
    wqT = np.ascontiguousarray(w_q.T)     # [D, 2048]
    wkT = np.ascontiguousarray(w_k.T)     # [D, 512]
    wvT = np.ascontiguousarray(w_v.T)
    woT = np.ascontiguousarray(w_o.T)     # [D(feat-in), D(out)]

    ind = np.kron(np.eye(16), np.ones((1, 64))).astype(np.float32)
    idn = np.eye(128, dtype=np.float32)

    in_maps = []
    for c in range(NCORES):
        b, j = divmod(c, 4)
        in_maps.append({
            "qT": qT[b],
            "kT": kTb[b],
            "vT": vTb[b],
            "wqT": np.ascontiguousarray(wqT[:, FEAT * j:FEAT * (j + 1)]),
            "wkT": np.ascontiguousarray(wkT[:, 128 * j:128 * (j + 1)]),
            "wvT": np.ascontiguousarray(wvT[:, 128 * j:128 * (j + 1)]),
            "woT": np.ascontiguousarray(woT[FEAT * j:FEAT * (j + 1), :]),
            "ind": ind,
            "idn_f": idn,
            "idn_r": idn,
        })
    return in_maps


def kernel(query, key, value, w_q, w_k, w_v, w_o):
    from concourse.bass_utils import run_bass_kernel_spmd

    nc = _get_nc()
    in_maps = _prep_in_maps(query, key, value, w_q, w_k, w_v, w_o)
    res = run_bass_kernel_spmd(nc, in_maps, list(range(NCORES))).results

    attention_weights = np.empty((B, HQ, S, S), dtype=np.float32)
    output = np.zeros((B, S, D), dtype=np.float32)
    for c in range(NCORES):
        b, j = divmod(c, 4)
        attention_weights[b, NH * j:NH * (j + 1)] = res[c]["attn"]
        output[b] += res[c]["outp"]
    return output, attention_weights
